# revision 63
# baseline (speedup 1.0000x reference)
"""GCN dialogue manager kernel for 8 trn2 NeuronCores.

Math (reference):
    h   = x @ W_gcn
    deg = in_deg(dst) + 1   (self loops)
    dinv = rsqrt(deg)
    agg[d] = sum_{e:(s->d)} dinv[s]*dinv[d]*h[s] + dinv[d]^2*h[d] + b_gcn
    out = agg @ W_act + b_act

Kernel strategy (dst-sharded, edges partitioned by destination):
    out[d] = dinv[d] * (sum_{slots of d} x~[s]) @ (W_gcn@W_act) + (b_gcn@W_act + b_act)
    with x~ = dinv * x prescaled per node (host, with the fp16 conversion);
    the self-loop slot contributes x~[d] so the flush's dinv[d] completes
    dinv[d]^2.
  - 8 cores each own 6250 destination nodes (node -> core node%8), 49 dst
    blocks of 128.
  - Edges are bucketed by (core, dst-block, src-half) on the host (integer
    metadata only) into a static per-group tile schedule (ntile/mcq =
    cross-core max, data-driven). SWDGE descriptor generation (~2.2ns/desc
    across 4 queues) is the kernel's hard floor, so the GpSimd engine runs
    ONLY dma_gather calls: per-slot fp16 x~ rows (256B) from two half-tables.
  - Self-loop rows arrive via a static strided DMA from a per-core xself
    table (host-sliced x~16[c::8]) into one dedicated column per dst block:
    zero SWDGE descriptors, prebuilt ring.
  - sel (0/1 one-hot slot->dst routing) is built on DVE with one broadcast
    tensor_tensor is_equal per chunk (~150ns/col) — runs under the gather
    floor; no weights in sel, so no second pass.
  - Aggregation per block: matmul(acc, lhsT=xg_col, rhs=sel_col) accumulated
    in PSUM -> acc[F, dst] (feature-major: no transpose needed downstream).
  - Flush per chunk of 4 blocks: ACT copies acc PSUM->SBUF, per-block matmul
    with fused W = W_gcn@W_act into a shared outp PSUM tile (dst-major), ACT
    scale-copy by dinv[d] per block, one DVE add of the broadcast fused
    bias, DMA out.
"""

import os
import sys

for _p in ("/opt/trn_rl_repo",):
    if _p not in sys.path and os.path.isdir(_p):
        sys.path.insert(0, _p)

import numpy as np

# ---- problem constants (hardcoded per spec) ----
N, E, F, HID, A = 50000, 600000, 128, 128, 64
P = 128                      # partitions
NCORE = 8
DST_PER_CORE = 6250          # N / 8
NBLK = 49                    # ceil(6250/128) dst blocks per core
OUT_ROWS = NBLK * P          # 6272 padded out rows per core
SELF_ROWS = OUT_ROWS         # xself table rows (6250 + 22 zero pad)
HALF = 25152                 # nodes [0,HALF) in table A, [HALF,...) in table B
XROWS = 25216                # rows per half table (HALF + 64 zero pad rows)
ZROW_A = 25152               # a zero row in table A (explicit pad row)
ZROW_B = 25024               # node 50176 -> xb row 25024 (zero: node >= N)
MAXTPG = 8                   # tiles per (block, half) group cap
XG_BUFS = 3                  # xg buffer rotation depth
CHUNK = 4                    # dst blocks per compute chunk
_CHUNKS = [(c * CHUNK, min(CHUNK, NBLK - c * CHUNK)) for c in range((NBLK + CHUNK - 1) // CHUNK)]

_prog_cache = {}


def _chunk_cols(ntile, b0, ncb):
    """Column layout of one chunk: per block [hA tiles][hB tiles], then one
    self column per block at the end. Returns (ncols, per-block col lists,
    self col base), all chunk-relative."""
    off = 0
    blocks = []
    for i in range(ncb):
        g = (b0 + i) * 2
        na, nb = int(ntile[g]), int(ntile[g + 1])
        blocks.append(list(range(off, off + na + nb)))
        off += na + nb
    scol0 = off
    for i in range(ncb):
        blocks[i].append(scol0 + i)
    return off + ncb, blocks, scol0


def _build_program(ntile, mcq):
    """Build the Bass program shared by all 8 cores.

    ntile: tuple of 98 ints — tiles (of 128 slots) per (block, half) group
    (edges only), group index g = blk*2 + half.
    mcq: tuple of 98 ints — gather count per group (cross-core max rounded
    up to 8; <= ntile*128). The gather's count register must equal the
    number of descriptors actually generated, so it is a shared constant
    preloaded at the prologue: each core pads its slots with zero-row idxs
    up to mcq[g] and idx=-1 beyond (skipped)."""
    key = (tuple(ntile), tuple(mcq))
    if key in _prog_cache:
        return _prog_cache[key]

    import concourse.bacc as bacc
    import concourse.mybir as mybir
    import concourse.tile as tile

    f32 = mybir.dt.float32
    bf16 = mybir.dt.float16  # message dtype: fp16 (10-bit mantissa, values are O(1))
    i32 = mybir.dt.int32
    i16 = mybir.dt.int16
    Alu = mybir.AluOpType
    Act = mybir.ActivationFunctionType

    # edge-slot columns (gathered); self columns live past them per chunk
    col_start = np.concatenate([[0], np.cumsum(ntile)]).astype(int)
    tot_col = int(col_start[-1])
    tot_slot = tot_col * P
    # sel metadata spans edge cols + self cols, chunk-concatenated
    chunk_meta = []
    m0 = 0
    for (b0, ncb) in _CHUNKS:
        ncols, blocks, scol0 = _chunk_cols(ntile, b0, ncb)
        chunk_meta.append((m0, ncols, blocks, scol0))
        m0 += ncols
    meta_cols = m0
    maxc = max(m[1] for m in chunk_meta)

    nc = bacc.Bacc(None, num_swdge_queues=4)

    xa = nc.dram_tensor("xa", [XROWS, F], bf16, kind="ExternalInput")
    xb = nc.dram_tensor("xb", [XROWS, F], bf16, kind="ExternalInput")
    xself = nc.dram_tensor("xself", [SELF_ROWS, F], bf16, kind="ExternalInput")
    idxs = nc.dram_tensor("idxs", [P, tot_slot // 16], i16, kind="ExternalInput")
    dstloc = nc.dram_tensor("dstloc", [P, meta_cols], bf16, kind="ExternalInput")
    dinvd = nc.dram_tensor("dinvd", [P, NBLK], f32, kind="ExternalInput")
    wgT = nc.dram_tensor("wgT", [HID, F], f32, kind="ExternalInput")
    wact = nc.dram_tensor("wact", [HID, A], f32, kind="ExternalInput")
    bgcn = nc.dram_tensor("bgcn", [HID, 1], f32, kind="ExternalInput")
    bact = nc.dram_tensor("bact", [1, A], f32, kind="ExternalInput")
    out = nc.dram_tensor("out", [OUT_ROWS, A], bf16, kind="ExternalOutput")

    with tile.TileContext(nc) as tc:
        with (
            tc.tile_pool(name="const", bufs=1) as cpool,
            tc.tile_pool(name="cpsum", bufs=1, space="PSUM") as cpsum,
            tc.tile_pool(name="sel", bufs=3) as spool,
            tc.tile_pool(name="acc", bufs=3, space="PSUM") as accpool,
            tc.tile_pool(name="outp", bufs=2, space="PSUM") as outppool,
            tc.tile_pool(name="flush", bufs=4) as fpool,
        ):
            # ---- constants / prologue ----
            # idx table loaded into one tile PER CHUNK: tile-granular
            # dependency tracking would otherwise gate the first gather on
            # all 13 slice loads instead of just its own
            idx_tiles = []
            for ci, (b0, ncb) in enumerate(_CHUNKS):
                a = int(col_start[b0 * 2]) * P // 16
                z = int(col_start[(b0 + ncb) * 2]) * P // 16
                idx_t = cpool.tile([P, z - a], i16, tag=f"idx{ci}")
                nc.sync.dma_start(out=idx_t[:], in_=idxs[:, a:z])
                idx_tiles.append((idx_t, a))

            # persistent gather buffers, rotated across chunks. Only the pad
            # slots (beyond mcq per group) need zeroing — real slots are
            # overwritten by the gather and stale finite data is masked by
            # sel=0 — so each chunk zeroes just its pad tails (tiny DVE ops)
            # instead of whole-buffer memsets gating the first gather.
            xg_bufs = []
            for _bi in range(XG_BUFS):
                xg_b = cpool.tile([P, maxc, F], bf16, tag=f"xgb{_bi}")
                xg_bufs.append(xg_b)

            def emit_pad_memsets(ci):
                b0, ncb = _CHUNKS[ci]
                c0 = int(col_start[b0 * 2])
                xg = xg_bufs[ci % XG_BUFS]
                for i in range(ncb):
                    for h in (0, 1):
                        g = (b0 + i) * 2 + h
                        nt = int(ntile[g])
                        m = int(mcq[g])
                        crel = int(col_start[g]) - c0
                        # whole columns from the mcq boundary: the gather is
                        # ordered after this and rewrites the real slots in
                        # the boundary column
                        ft = crel + m // P
                        if ft < crel + nt:
                            nc.vector.memset(xg[:, ft:crel + nt, :], 0.0)

            # first use of each buffer: emit its pad memsets ahead of the
            # weights prologue so the first gathers aren't gated behind it
            for _ci in range(min(XG_BUFS, len(_CHUNKS))):
                emit_pad_memsets(_ci)
            num_regs = {int(v): nc.gpsimd.to_reg(int(v))
                        for v in sorted(set(int(t) for t in mcq))}
            qn = [0]

            def emit_gathers(ci):
                b0, ncb = _CHUNKS[ci]
                c0 = int(col_start[b0 * 2])
                scol0 = chunk_meta[ci][3]
                xg = xg_bufs[ci % XG_BUFS]
                idx_t, abase = idx_tiles[ci]
                for i in range(ncb):
                    for h, tab in ((0, xa), (1, xb)):
                        g = (b0 + i) * 2 + h
                        nt = int(ntile[g])
                        num = nt * P
                        crel = int(col_start[g]) - c0
                        s0 = int(col_start[g]) * P
                        nc.gpsimd.dma_gather(
                            xg[:, crel: crel + nt, :],
                            tab[:],
                            idx_t[:, s0 // 16 - abase: (s0 + num) // 16 - abase],
                            num,
                            num_regs[int(mcq[g])],
                            F,
                            queue_num=qn[0] % 4,
                        )
                        qn[0] += 1
                # self-loop rows: static strided DMA, no SWDGE descriptors
                nc.sync.dma_start(
                    out=xg[:, scol0:scol0 + ncb, :],
                    in_=xself[b0 * P:(b0 + ncb) * P, :].rearrange(
                        "(i p) f -> p i f", p=P),
                )

            for _bi in range(XG_BUFS):
                nc.vector.memset(xg_bufs[_bi][:], 0.0)

            # per-slot routing metadata
            dstloc_sb = cpool.tile([P, meta_cols], bf16)
            nc.sync.dma_start(out=dstloc_sb[:], in_=dstloc[:])

            # per-dst-block rsqrt(deg) for the flush scale
            dinvd_sb = cpool.tile([P, NBLK], f32)
            nc.sync.dma_start(out=dinvd_sb[:], in_=dinvd[:])

            iota_i = cpool.tile([P, P], i32)
            nc.gpsimd.iota(iota_i[:], pattern=[[1, P]], base=0, channel_multiplier=0)
            iota_bf = cpool.tile([P, P], bf16)
            nc.vector.tensor_copy(out=iota_bf[:], in_=iota_i[:])

            wgT_sb = cpool.tile([HID, F], f32)
            nc.sync.dma_start(out=wgT_sb[:], in_=wgT[:])
            wact_sb = cpool.tile([HID, A], f32)
            nc.sync.dma_start(out=wact_sb[:], in_=wact[:])
            wf_ps = cpsum.tile([F, A], f32, space="PSUM", tag="cps")
            nc.tensor.matmul(wf_ps[:], lhsT=wgT_sb[:], rhs=wact_sb[:], start=True, stop=True)
            wf_sb = cpool.tile([F, A], f32)
            nc.vector.tensor_copy(out=wf_sb[:], in_=wf_ps[:])

            bgcn_sb = cpool.tile([HID, 1], f32)
            nc.sync.dma_start(out=bgcn_sb[:], in_=bgcn[:])
            bact_sb = cpool.tile([1, A], f32)
            nc.sync.dma_start(out=bact_sb[:], in_=bact[:])
            cb_ps = cpsum.tile([1, A], f32, space="PSUM", tag="cps")
            nc.tensor.matmul(cb_ps[:], lhsT=bgcn_sb[:], rhs=wact_sb[:], start=True, stop=True)
            cb_sb = cpool.tile([1, A], f32)
            nc.vector.tensor_copy(out=cb_sb[:], in_=cb_ps[:])
            nc.vector.tensor_tensor(out=cb_sb[:], in0=cb_sb[:], in1=bact_sb[:], op=Alu.add)
            ones_row = cpool.tile([1, P], f32)
            nc.vector.memset(ones_row[:], 1.0)
            # fused bias broadcast to all partitions, tiled per chunk block
            cbr_ps = cpsum.tile([P, A], f32, space="PSUM", tag="cbr")
            nc.tensor.matmul(cbr_ps[:], lhsT=ones_row[:], rhs=cb_sb[:], start=True, stop=True)
            cb_rep = cpool.tile([P, CHUNK, A], f32)
            for i in range(CHUNK):
                nc.vector.tensor_copy(out=cb_rep[:, i, :], in_=cbr_ps[:])

            # ---- main loop over chunks of dst blocks ----
            for ci, (b0, ncb) in enumerate(_CHUNKS):
                m0, ncols, blocks, scol0 = chunk_meta[ci]
                xg = xg_bufs[ci % XG_BUFS]
                if ci >= XG_BUFS:
                    emit_pad_memsets(ci)
                emit_gathers(ci)
                # sel: 0/1 one-hot routing matrix, one broadcast is_equal
                sel = spool.tile([P, maxc, P], bf16, tag="sel")
                nc.vector.tensor_tensor(
                    out=sel[:, :ncols, :],
                    in0=dstloc_sb[:, m0:m0 + ncols].unsqueeze(2).broadcast_to([P, ncols, P]),
                    in1=iota_bf[:].unsqueeze(1).broadcast_to([P, ncols, P]),
                    op=Alu.is_equal,
                )
                outp = outppool.tile([P, CHUNK, A], f32, space="PSUM", tag="outp")
                for i in range(ncb):
                    # acc[F, dst] = sum_cols xg_col^T @ sel_col  (PSUM accum)
                    acc = accpool.tile([P, P], f32, space="PSUM", tag="acc")
                    cols = blocks[i]
                    for j, col in enumerate(cols):
                        nc.tensor.matmul(
                            acc[:],
                            lhsT=xg[:, col, :],
                            rhs=sel[:, col, :],
                            start=(j == 0),
                            stop=(j == len(cols) - 1),
                        )
                    # flush block: acc is feature-major, no transpose needed
                    accS = fpool.tile([P, P], f32, tag="accS")
                    nc.scalar.activation(accS[:], acc[:], Act.Copy)
                    nc.tensor.matmul(outp[:, i, :], lhsT=accS[:], rhs=wf_sb[:], start=True, stop=True)
                # per-block dinv[d] scale (outp is dst-major: per-partition
                # ACT scale), then bias add for the whole chunk in one DVE op
                out_sc = fpool.tile([P, CHUNK, A], f32, tag="outsc")
                for i in range(ncb):
                    b = b0 + i
                    nc.scalar.activation(out_sc[:, i, :], outp[:, i, :], Act.Copy,
                                         scale=dinvd_sb[:, b:b + 1])
                out_sb = fpool.tile([P, CHUNK, A], bf16, tag="outs")
                nc.vector.tensor_tensor(out=out_sb[:, :ncb, :], in0=out_sc[:, :ncb, :],
                                        in1=cb_rep[:, :ncb, :], op=Alu.add)
                for i in range(ncb):
                    b = b0 + i
                    nc.sync.dma_start(out=out[b * P:(b + 1) * P, :], in_=out_sb[:, i, :])

    nc.compile()
    _prog_cache[key] = nc
    return nc


def _preprocess(x, edge_index):
    """Host-side sharding: bucket edges by (core, dst block, src half) and
    build the static padded slot arrays. Integer/layout work only."""
    src = np.asarray(edge_index[0], dtype=np.int64)
    dst = np.asarray(edge_index[1], dtype=np.int64)

    in_deg = np.bincount(dst, minlength=N).astype(np.int64)
    deg_tot = in_deg + 1  # self loop

    # slots: real edges only (self loops arrive via the static xself DMA)
    s_src = src
    s_dst = dst

    # strided dst sharding (node -> core node%8)
    core = s_dst % NCORE
    loc = s_dst // NCORE
    blk = loc >> 7
    dloc = loc & 127
    half = (s_src >= HALF).astype(np.int64)
    rowid = s_src - HALF * half

    # group = (core, blk, half); position within group via stable sort
    g = (core * NBLK + blk) * 2 + half
    order = np.argsort(g, kind="stable")
    g_sorted = g[order]
    cnt = np.bincount(g_sorted, minlength=NCORE * NBLK * 2)
    # static tile schedule: cross-core max per (blk, half) group
    cnt2 = cnt.reshape(NCORE, NBLK * 2)
    ntile = np.maximum(1, -(-cnt2.max(axis=0) // P))  # [98]
    if ntile.max() > MAXTPG:
        raise RuntimeError(f"group needs {ntile.max()} tiles > {MAXTPG}")
    col_start = np.concatenate([[0], np.cumsum(ntile)]).astype(np.int64)
    tot_col = int(col_start[-1])
    tot_slot = tot_col * P

    starts = np.zeros_like(cnt)
    starts[1:] = np.cumsum(cnt)[:-1]
    pos_in_group = np.arange(len(order)) - starts[g_sorted]

    blk_s = blk[order]
    half_s = half[order]
    g2 = blk_s * 2 + half_s
    col = col_start[g2] + (pos_in_group >> 7)
    p = pos_in_group & 127
    flat = col * P + p  # gathered slot id within core

    core_s = core[order]
    rowid_s = rowid[order]
    dloc_s = dloc[order]

    # Pad structure per (core, group): [real slots | ZROW pads up to mcq[g]
    # | idx=-1]. mcq = cross-core max count rounded up to 8. The gather
    # generates exactly mcq[g] descriptors on every core (trailing -1s are
    # trimmed by the ucode); sel=0 masks all pad rows.
    mcq = np.minimum(-(-cnt2.max(axis=0) // 8) * 8, ntile * P).astype(np.int64)
    colg = np.repeat(np.arange(NBLK * 2), ntile)          # group of each column
    slotg = np.repeat(colg, P)                            # group of each slot
    g_off = np.concatenate([[0], np.cumsum(ntile * P)])   # slot base per group
    in_mcq = (np.arange(tot_slot) - g_off[slotg]) < mcq[slotg]
    zrow = np.where(slotg % 2 == 1, ZROW_B, ZROW_A).astype(np.int16)
    idx_arr = np.empty((NCORE, tot_slot), dtype=np.int16)
    idx_arr[:] = np.where(in_mcq, zrow, np.int16(-1))[None, :]
    dst_arr = np.full((NCORE, tot_slot), -1.0, dtype=np.float16)

    lin = core_s * tot_slot + flat
    idx_arr.reshape(-1)[lin] = rowid_s.astype(np.int16)
    dst_arr.reshape(-1)[lin] = dloc_s.astype(np.float16)

    # idxs: 16-partition wrap replicated 8x -> [128, tot_slot//16]
    idx_wrap = idx_arr.reshape(NCORE, tot_slot // 16, 16).transpose(0, 2, 1)
    idx_rep = np.tile(idx_wrap, (1, 8, 1)).copy()

    dst_pc = dst_arr.reshape(NCORE, tot_col, P).transpose(0, 2, 1)

    # sel metadata in chunk-concatenated layout: per chunk the gathered
    # edge cols, then one self col per block (dst = diagonal)
    meta_cols = tot_col + NBLK
    dst_meta = np.full((NCORE, P, meta_cols), -1.0, dtype=np.float16)
    node = np.arange(N, dtype=np.int64)
    has_node = np.zeros((NCORE, OUT_ROWS), dtype=bool)
    has_node[node % NCORE, node // NCORE] = True
    m0 = 0
    for (b0, ncb) in _CHUNKS:
        c0, c1 = int(col_start[b0 * 2]), int(col_start[(b0 + ncb) * 2])
        gcols = c1 - c0
        dst_meta[:, :, m0:m0 + gcols] = dst_pc[:, :, c0:c1]
        for i in range(ncb):
            b = b0 + i
            sc = m0 + gcols + i
            l = b * P + np.arange(P)
            ok = has_node[:, l]                                # [NCORE, P]
            dst_meta[:, :, sc] = np.where(ok, np.arange(P)[None, :], -1.0)
        m0 += gcols + ncb

    # per-dst-block rsqrt(deg) for the flush scale (1.0 on pad rows)
    dinv = (1.0 / np.sqrt(deg_tot.astype(np.float64))).astype(np.float32)
    dinvd = np.ones((NCORE, P, NBLK), dtype=np.float32)
    nc_ = node % NCORE
    nl = node // NCORE
    dinvd[nc_, nl & 127, nl >> 7] = dinv

    # prescaled x~ = dinv * x, fp16 half tables (zero padded): 256B rows
    x16 = (np.asarray(x, dtype=np.float32) * dinv[:, None]).astype(np.float16)
    xa = np.zeros((XROWS, F), dtype=np.float16)
    xa[:HALF] = x16[:HALF]
    xb = np.zeros((XROWS, F), dtype=np.float16)
    xb[: N - HALF] = x16[HALF:]
    # per-core self tables: x~16[c::8] padded to SELF_ROWS
    xself = np.zeros((NCORE, SELF_ROWS, F), dtype=np.float16)
    for c in range(NCORE):
        rows = x16[c::NCORE]
        xself[c, :rows.shape[0]] = rows

    return (ntile, mcq, xa, xb, xself, idx_rep,
            np.ascontiguousarray(dst_meta), dinvd)


def kernel(x, edge_index, W_gcn, b_gcn, W_act, b_act):
    from concourse.bass_utils import run_bass_kernel_spmd

    x = np.ascontiguousarray(np.asarray(x, dtype=np.float32))
    (ntile, mcq, xa, xb, xself, idx_rep, dst_meta, dinvd) = _preprocess(x, edge_index)

    wgT = np.ascontiguousarray(np.asarray(W_gcn, dtype=np.float32).T)
    wact = np.ascontiguousarray(np.asarray(W_act, dtype=np.float32))
    bg = np.ascontiguousarray(np.asarray(b_gcn, dtype=np.float32).reshape(HID, 1))
    ba = np.ascontiguousarray(np.asarray(b_act, dtype=np.float32).reshape(1, A))

    nc = _build_program(tuple(int(v) for v in ntile), tuple(int(v) for v in mcq))
    in_maps = [
        {
            "xa": xa,
            "xb": xb,
            "xself": xself[c],
            "idxs": idx_rep[c],
            "dstloc": dst_meta[c],
            "dinvd": dinvd[c],
            "wgT": wgT,
            "wact": wact,
            "bgcn": bg,
            "bact": ba,
        }
        for c in range(NCORE)
    ]
    trace = bool(os.environ.get("GCN_TRACE"))
    res = run_bass_kernel_spmd(nc, in_maps, core_ids=list(range(NCORE)), trace=trace)
    kernel.last_results = res

    out = np.empty((N, A), dtype=np.float32)
    for c in range(NCORE):
        out[c::NCORE] = res.results[c]["out"][:DST_PER_CORE].astype(np.float32)
    return out
